# revision 43
# baseline (speedup 1.0000x reference)
"""Sparse span-attention kernel for Trainium2 (8 NeuronCores, SPMD).

Math (matches the reference):
  - Only the CLS query row of the MHA survives downstream, and K/V are
    shared by all spans of a sequence. Per batch we precompute
    P[t,h] = exp(score[t,h]) and WV[t,j] = P[t,head(j)] * v[t,j] once,
    then each span's softmax context is a masked row-sum:
      ctx[n] = (G_cls + sum_{t in span n} G[t]) split into num/den.
    The masked row-sum over 512 token positions is a mask matmul
    (mask built on VectorE from span start/end via iota compares).
  - out_proj is folded into w1 (host-side weight fusion); the width-
    embedding contribution becomes a padded [128,3072] table applied via
    a width-one-hot matmul; the cls_reps contribution is a per-batch bias.

Span dedup: a span's output depends only on (start, width, mask) — and
for masked-out spans on width alone. Random spans collide heavily
(~2420 unique of 4096 per batch), so the host dedups spans per batch,
the device computes only unique spans, and the host expands the result
back with a gather. Per-core capacity is 1280 unique spans (blocks of
512/512/256); a 2048-capacity variant is compiled on demand if an input
ever has more unique spans than that.

Sharding: core c handles batch c//2 and half c%2 of that batch's
deduped span list. No collectives: each core writes its own output
shard; the host gathers.
"""

import math

import numpy as np
import ml_dtypes

import concourse.bass as bass
import concourse.mybir as mybir
from concourse.bass import ts
from concourse.tile import TileContext
from concourse.vector_clock import ScopedClock

F32 = mybir.dt.float32
F16 = mybir.dt.float16
BF16 = mybir.dt.bfloat16
bf = ml_dtypes.bfloat16
ALU = mybir.AluOpType
ACTF = mybir.ActivationFunctionType

B, S, H, NH, MAXW = 4, 512, 768, 4, 8
DH = H // NH                # 192
N = S * MAXW                # 4096 spans per batch
INNER = 3072
WD = 64
SCALE = 1.0 / math.sqrt(DH)
KC = H // 128               # 6 contraction chunks of 128 over hidden
OC = INNER // 128           # 24 chunks over inner dim
GC = S // 128               # 4 token chunks

def _blocks_for(cap):
    """Split a span capacity into matmul-friendly blocks of <=512."""
    blks = []
    while cap > 512:
        blks.append(512)
        cap -= 512
    blks.append(cap)
    return tuple(blks)

# ---------------------------------------------------------------------------
# walrus workaround: this build rejects >1 sync wait per instruction.
# Hoist extra waits onto standalone EventSemaphore instructions.
# ---------------------------------------------------------------------------
_orig_commit = TileContext._commit_instruction


def _split_waits(self, inst):
    si = inst.sync_info
    waits = list(si.on_wait)
    for w in waits[:-1]:
        ev = mybir.InstEventSemaphore(
            name=self.nc.get_next_instruction_name(),
            engine=inst.engine,
            ins=[],
            outs=[],
            sync_info=mybir.SyncInfo(on_wait=[w], on_update=[]),
        )
        self._add_instruction(ev)
    inst.sync_info = mybir.SyncInfo(on_wait=[waits[-1]], on_update=list(si.on_update))


def _patched_commit(self, inst, lazy_reg_writes=True):
    if (
        inst.engine != mybir.EngineType.Unassigned
        and inst.sync_info is not None
        and len(inst.sync_info.on_wait) > 1
    ):
        _split_waits(self, inst)
    return _orig_commit(self, inst, lazy_reg_writes)


def _patched_drain_and_barrier(self, tick_clock, wait_clock):
    nc = self.nc
    probe = nc.sync.drain()
    wait_clock.add_sem_waits(probe.ins, ScopedClock({None: tick_clock.global_clock}))
    waits = list(probe.ins.sync_info.on_wait)
    probe.ins.sync_info = mybir.SyncInfo(on_wait=[], on_update=[])
    for w in waits:
        ev = mybir.InstEventSemaphore(
            name=nc.get_next_instruction_name(),
            engine=mybir.EngineType.SP,
            ins=[],
            outs=[],
            sync_info=mybir.SyncInfo(on_wait=[w], on_update=[]),
        )
        nc.register_instruction(ev, overwrite=True)
        nc.cur_bb.bb.add_instruction(ev)
    nc.sync.drain()

    nc.all_engine_barrier()
    assert self.sems is not None
    popped = nc._tile_sem_poison_stack.pop()
    assert popped is self._sem_poison
    nc.clear_and_free_semaphores(list(self.sems.allocated().values()))


def _install_patches():
    TileContext._commit_instruction = _patched_commit
    TileContext._drain_and_barrier = _patched_drain_and_barrier


_install_patches()


# ---------------------------------------------------------------------------
# device graph
# ---------------------------------------------------------------------------
def build(blks):
    nspc = sum(blks)
    nblk = len(blks)

    nc = bass.Bass("TRN2")

    d_xT = nc.dram_tensor("xT", [128, KC * S], BF16, kind="ExternalInput")
    d_wvT = nc.dram_tensor("wvT", [128, KC * H], BF16, kind="ExternalInput")
    d_rhT = nc.dram_tensor("rhT", [128, KC * NH], BF16, kind="ExternalInput")
    d_stbc = nc.dram_tensor("stbc", [128, nspc], F16, kind="ExternalInput")
    d_enbc = nc.dram_tensor("enbc", [128, nspc], F16, kind="ExternalInput")
    d_wdbc = nc.dram_tensor("wdbc", [128, nspc], F16, kind="ExternalInput")
    d_bvbc = nc.dram_tensor("bvbc", [128, H], F32, kind="ExternalInput")
    d_scols = nc.dram_tensor("scols", [128, GC], F32, kind="ExternalInput")
    d_kb = nc.dram_tensor("kb", [128, NH], F32, kind="ExternalInput")
    d_gclscol = nc.dram_tensor("gclscol", [128, KC], F32, kind="ExternalInput")
    d_pclscol = nc.dram_tensor("pclscol", [NH, 1], F32, kind="ExternalInput")
    d_weffT = nc.dram_tensor("weffT", [128, KC * INNER], BF16, kind="ExternalInput")
    d_clscol = nc.dram_tensor("clscol", [128, OC], F32, kind="ExternalInput")
    d_tcT = nc.dram_tensor("tcT", [128, INNER], BF16, kind="ExternalInput")
    d_w2T = nc.dram_tensor("w2T", [128, OC * H], BF16, kind="ExternalInput")
    d_b2bc = nc.dram_tensor("b2bc", [128, H], F32, kind="ExternalInput")
    d_out = nc.dram_tensor("out", [nspc, H], BF16, kind="ExternalOutput")

    with TileContext(nc) as tc:
        with tc.tile_pool(name="const", bufs=1) as cp, \
             tc.tile_pool(name="work", bufs=1) as wp, \
             tc.tile_pool(name="sbM", bufs=2) as sbM, \
             tc.tile_pool(name="sbB", bufs=2) as sbB, \
             tc.tile_pool(name="sbB1", bufs=1) as sbB1, \
             tc.tile_pool(name="sbO", bufs=2) as sbO, \
             tc.tile_pool(name="dramp", bufs=2, space="DRAM") as dramp:
            # ---- warmup tile: vector-memset so the PE can start ramping
            # immediately, with no DMA dependency
            wu_sb = cp.tile([128, 512], BF16)
            nc.vector.memset(wu_sb[:], 0.0)

            # ---- stage-A inputs (sync/HWDGE queues). Queue order matters:
            # xT first (head of the dependency chain — every small DMA ahead
            # of it costs queue time), then the small constants, then the V
            # weights.
            xt = cp.tile([128, KC, S], BF16)
            nc.sync.dma_start(xt[:], d_xT.rearrange("p (k s) -> p k s", k=KC))
            rh_sb = cp.tile([128, KC, NH], BF16)
            nc.sync.dma_start(rh_sb[:], d_rhT.rearrange("p (k h) -> p k h", k=KC))
            scols_sb = cp.tile([128, GC], F32)
            nc.sync.dma_start(scols_sb[:], d_scols[:])
            kb_sb = cp.tile([128, NH], F32)
            nc.sync.dma_start(kb_sb[:], d_kb[:])
            gclscol_sb = cp.tile([128, KC], F32)
            nc.sync.dma_start(gclscol_sb[:], d_gclscol[:])
            pclscol_sb = cp.tile([NH, 1], F32)
            nc.sync.dma_start(pclscol_sb[:], d_pclscol[:])
            clscol_sb = cp.tile([128, OC], F32)
            nc.sync.dma_start(clscol_sb[:], d_clscol[:])

            # span bounds, host-pre-broadcast to 128 partitions in f16, on
            # the Activation HWDGE queue (independent of the SP queue) so
            # the block-0 masks never wait behind the main input stream
            stbc_sb = cp.tile([128, nspc], F16)
            nc.scalar.dma_start(stbc_sb[:], d_stbc[:])
            enbc_sb = cp.tile([128, nspc], F16)
            nc.scalar.dma_start(enbc_sb[:], d_enbc[:])
            wdbc_sb = cp.tile([128, nspc], F16)
            nc.scalar.dma_start(wdbc_sb[:], d_wdbc[:])
            bvbc_sb = cp.tile([128, H], F32)
            nc.scalar.dma_start(bvbc_sb[:], d_bvbc[:])

            # ---- per-block mask tiles; block k+1's masks are built while
            # block k computes, block 0's before the weights hit the queues
            MT_all = [wp.tile([128, GC, blks[b_]], BF16, tag=f"mt{b_}",
                              name=f"mt{b_}")
                      for b_ in range(nblk)]
            OH_all = [wp.tile([128, blks[b_]], BF16, tag=f"oh{b_}",
                              name=f"oh{b_}")
                      for b_ in range(nblk)]

            def emit_masks(blk):
                bs = blks[blk]
                n0 = sum(blks[:blk])
                st_bc = stbc_sb[:, n0:n0 + bs]
                en_bc = enbc_sb[:, n0:n0 + bs]
                wd_bc = wdbc_sb[:, n0:n0 + bs]
                for c in range(GC):
                    tmp = sbM.tile([128, bs], F16, tag="tmp", name="tmp")
                    nc.vector.tensor_scalar(tmp[:], st_bc,
                                            scols_sb[:, c:c + 1], None,
                                            ALU.is_le)
                    nc.vector.scalar_tensor_tensor(
                        MT_all[blk][:, c, :], en_bc,
                        scols_sb[:, c:c + 1], tmp[:],
                        ALU.is_gt, ALU.mult)
                nc.vector.tensor_scalar(OH_all[blk][:], wd_bc,
                                        scols_sb[:, 0:1], None, ALU.is_equal)

            emit_masks(0)

            # V weights queue after the block-0 mask broadcasts
            wv_t = [cp.tile([128, H], BF16, tag=f"wv{k}", name=f"wv{k}")
                    for k in range(KC)]
            for k in range(KC):
                nc.sync.dma_start(wv_t[k][:], d_wvT[:, k * H:(k + 1) * H])
            # b2 broadcast is only needed by the FFN2 epilogue; queue it last
            b2bc_sb = cp.tile([128, H], F32)
            for j0 in (0, 384):
                nc.sync.dma_start(b2bc_sb[:, j0:j0 + 384],
                                  d_b2bc[:, j0:j0 + 384])

            G = wp.tile([128, GC, H + NH], BF16)       # [token, chunk, WV|P]
            G_p = wp.tile([128, GC, NH], F32)          # raw P per token chunk

            # partition ranges of each hidden chunk -> head row
            RB_PIECES = ((0, 0, 128, 0), (1, 0, 64, 0), (1, 64, 128, 1),
                         (2, 0, 128, 1), (3, 0, 128, 2), (4, 0, 64, 2),
                         (4, 64, 128, 3), (5, 0, 128, 3))

            def emit_den_recip(blk, dpool):
                bs = blks[blk]
                MT = MT_all[blk]
                ps_d = dpool.tile([128, bs], F32, tag="d", name="d")
                for c in range(GC):
                    nc.tensor.matmul(ps_d[0:NH, :], G[:, c, H:H + NH],
                                     MT[:, c, :], start=(c == 0),
                                     stop=(c == GC - 1))
                den_sb = sbB.tile([NH, bs], F32, tag="den", name="den")
                nc.vector.tensor_scalar(den_sb[:], ps_d[0:NH, :],
                                        pclscol_sb[0:NH, 0:1], None, ALU.add)
                rec = sbB.tile([NH, bs], F16, tag="rc", name="rc")
                with nc.allow_low_precision(reason="f16 recip: 0.05% rel err, "
                                            "well inside the 2e-2 gate"):
                    nc.vector.reciprocal(rec[:], den_sb[:])
                # broadcast per-head reciprocals to the 768 hidden rows via
                # a DRAM round-trip (SBUF APs cannot partition-broadcast);
                # f16 halves the round-trip bytes
                scr = dramp.tile([NH, bs], F16, tag="rsc", name="rsc")
                nc.scalar.dma_start(scr[:], rec[:])
                rb = [sbB1.tile([128, bs], F16, tag=f"rb{c}", name=f"rb{c}")
                      for c in range(KC)]
                for (c, p0, p1, h) in RB_PIECES:
                    nc.scalar.dma_start(
                        rb[c][p0:p1, :],
                        scr[h:h + 1, :].to_broadcast((p1 - p0, bs)))
                return rb

            rbs = {}

            # ---- big weights: SWDGE queues, split per chunk so transfers
            # parallelize across rings and FFN1's k-chunk matmuls depend only
            # on their own slice; gated on the first xT chunk landing so they
            # don't starve the stage-A input DMAs of HBM bandwidth
            weff_sb = cp.tile([128, KC, INNER], BF16)
            for k in range(KC):
                nc.vector.tensor_copy(weff_sb[0:1, k, 0:1], wu_sb[0:1, 0:1])
                nc.gpsimd.dma_start(weff_sb[:, k, :],
                                    d_weffT[:, k * INNER:(k + 1) * INNER])
            tc_sb = cp.tile([128, INNER], BF16)
            nc.vector.tensor_copy(tc_sb[0:1, 0:1], xt[0:1, 0, 0:1])
            nc.gpsimd.dma_start(tc_sb[:], d_tcT[:])
            w2_sb = cp.tile([128, OC, H], BF16)
            for o in range(OC):
                nc.vector.tensor_copy(w2_sb[0:1, o, 0:1], xt[0:1, 0, 0:1])
                nc.gpsimd.dma_start(w2_sb[:, o, :],
                                    d_w2T[:, o * H:(o + 1) * H])

            # ---- stage A: warmup burst on the memset tile (ramp the PE
            # while input DMAs land), scores/exp/P, V projections + WV
            # scaling, then block-0 denominator (after V so the PE never
            # waits on the vector engine's masks/P)
            with tc.tile_pool(name="psA", bufs=1, space="PSUM") as psA, \
                 tc.tile_pool(name="psAv", bufs=3, space="PSUM") as psAv, \
                 tc.tile_pool(name="psA1", bufs=1, space="PSUM") as psA1:
                ps_wu = psAv.tile([128, H], F32, tag="v")
                for _ in range(12):
                    nc.tensor.matmul(ps_wu[:, 0:512], wu_sb[:, 0:128],
                                     wu_sb[:, 0:512], start=True, stop=True)
                ps_sc = psA.tile([128, GC, NH], F32, tag="sc")
                for c in range(GC):
                    for k in range(KC):
                        nc.tensor.matmul(ps_sc[:, c, :], xt[:, k, ts(c, 128)],
                                         rh_sb[:, k, :],
                                         start=(k == 0), stop=(k == KC - 1))
                    nc.scalar.activation(G_p[:, c, :], ps_sc[:, c, :], ACTF.Exp,
                                         scale=SCALE)
                    nc.vector.tensor_tensor(G[:, c, H:H + NH], G_p[:, c, :],
                                            kb_sb[:, :], ALU.mult)
                # block-0 denominator between scores and V: the recip DRAM
                # round-trip hides under the V projections
                rbs[0] = emit_den_recip(0, psA1)
                for c in range(GC):
                    ps_v = psAv.tile([128, H], F32, tag="v")
                    for f0, fw in ((0, 512), (512, 256)):
                        for k in range(KC):
                            nc.tensor.matmul(ps_v[:, f0:f0 + fw],
                                             xt[:, k, ts(c, 128)],
                                             wv_t[k][:, f0:f0 + fw],
                                             start=(k == 0),
                                             stop=(k == KC - 1))
                    # V bias folded in on the vector engine (a contraction-1
                    # matmul costs ~2.5x a normal row on the PE)
                    nc.vector.tensor_tensor(ps_v[:, :], ps_v[:, :],
                                            bvbc_sb[:, :], ALU.add)
                    for h in range(NH):
                        nc.vector.tensor_scalar_mul(
                            G[:, c, ts(h, DH)], ps_v[:, ts(h, DH)],
                            G_p[:, c, h:h + 1])

            # ---- stage B: per span block
            with tc.tile_pool(name="psD", bufs=1, space="PSUM") as psD, \
                 tc.tile_pool(name="psN", bufs=3, space="PSUM") as psN, \
                 tc.tile_pool(name="psH", bufs=2, space="PSUM") as psH, \
                 tc.tile_pool(name="psO", bufs=2, space="PSUM") as psO:
                for blk in range(nblk):
                    bs = blks[blk]
                    n0 = sum(blks[:blk])
                    MT = MT_all[blk]
                    OH = OH_all[blk][:]
                    rb_t = rbs.pop(blk)

                    # numerators + recip broadcast + divide (CLS via epilogue)
                    ctx_t = [sbB1.tile([128, bs], BF16, tag=f"ctx{c}",
                                       name=f"ctx{c}")
                             for c in range(KC)]
                    ps_ns = {}

                    def emit_num(c):
                        ps_n = psN.tile([128, bs], F32, tag="n")
                        for cc in range(GC):
                            nc.tensor.matmul(ps_n[:], G[:, cc, ts(c, 128)],
                                             MT[:, cc, :], start=(cc == 0),
                                             stop=(cc == GC - 1))
                        ps_ns[c] = ps_n

                    def emit_div(c):
                        nc.vector.scalar_tensor_tensor(
                            ctx_t[c][:], ps_ns.pop(c),
                            gclscol_sb[:, c:c + 1], rb_t[c][:],
                            ALU.add, ALU.mult)

                    emit_num(0)
                    emit_num(1)
                    emit_num(2)
                    for c in range(KC):
                        emit_div(c)
                        if c + 3 < KC:
                            emit_num(c + 3)
                    if blk + 1 < nblk:
                        emit_masks(blk + 1)

                    # FFN1 (out_proj folded in) + width table + cls bias, relu
                    h1_t = [sbB1.tile([128, bs], BF16, tag=f"h1_{o}",
                                      name=f"h1_{o}")
                            for o in range(OC)]
                    for o in range(OC):
                        ps_h = psH.tile([128, bs], F32, tag="h")
                        for k in range(KC):
                            nc.tensor.matmul(ps_h[:], weff_sb[:, k, ts(o, 128)],
                                             ctx_t[k][:],
                                             start=(k == 0), stop=False)
                        nc.tensor.matmul(ps_h[:], tc_sb[:, ts(o, 128)],
                                         OH, start=False, stop=True)
                        nc.scalar.activation(h1_t[o][:], ps_h[:], ACTF.Relu,
                                             bias=clscol_sb[:, o:o + 1])

                    if blk + 1 < nblk:
                        rbs[blk + 1] = emit_den_recip(blk + 1, psD)

                    # FFN2 back to [span, hidden]; b2 added in the epilogue
                    for t in range((bs + 127) // 128):
                        tw = min(128, bs - t * 128)
                        out_sb = sbO.tile([128, H], BF16, tag="os")
                        for f0, fw in ((0, 512), (512, 256)):
                            ps_o = psO.tile([128, 512], F32, tag="o")
                            for k in range(OC):
                                nc.tensor.matmul(ps_o[0:tw, 0:fw],
                                                 h1_t[k][:, t * 128:t * 128 + tw],
                                                 w2_sb[:, k, f0:f0 + fw],
                                                 start=(k == 0),
                                                 stop=(k == OC - 1))
                            nc.vector.tensor_tensor(out_sb[0:tw, f0:f0 + fw],
                                                    ps_o[0:tw, 0:fw],
                                                    b2bc_sb[0:tw, f0:f0 + fw],
                                                    ALU.add)
                        nc.sync.dma_start(
                            d_out[n0 + t * 128:n0 + t * 128 + tw, :],
                            out_sb[0:tw, :])
    return nc


# ---------------------------------------------------------------------------
# host-side prep
# ---------------------------------------------------------------------------
def _dedup_batch(starts_b, widths_b, masks_b):
    """Unique (start,width,mask) combos (mask-0 spans key on width only).

    Returns (u_starts, u_widths, u_ends, inv) with inv mapping each of the
    N spans to its row in the unique list.
    """
    s = starts_b.astype(np.int64)
    w = widths_b.astype(np.int64)
    m = masks_b.astype(np.int64)
    key = np.where(m > 0, (1 << 20) + s * 16 + w, w)
    uniq, inv = np.unique(key, return_inverse=True)
    live = uniq >= (1 << 20)
    us = np.where(live, (uniq - (1 << 20)) >> 4, 0)
    uw = np.where(live, (uniq - (1 << 20)) & 15, uniq)
    ue = np.where(live, us + uw, 0)
    return us.astype(np.float32), uw.astype(np.float32), \
        ue.astype(np.float32), inv.reshape(-1)


def _prep_in_maps(token_reps, span_ids, span_masks, cls_reps, span_widths,
                  cls_embedding, in_proj_w, in_proj_b, out_proj_w, out_proj_b,
                  width_table, w1, b1, w2, b2):
    f32 = np.float32
    token_reps = np.asarray(token_reps, f32)
    span_ids = np.asarray(span_ids)
    span_masks = np.asarray(span_masks)
    cls_reps = np.asarray(cls_reps, f32)
    span_widths = np.asarray(span_widths)
    cls_embedding = np.asarray(cls_embedding, f32)
    in_proj_w = np.asarray(in_proj_w, f32)
    in_proj_b = np.asarray(in_proj_b, f32)
    out_proj_w = np.asarray(out_proj_w, f32)
    out_proj_b = np.asarray(out_proj_b, f32)
    width_table = np.asarray(width_table, f32)
    w1 = np.asarray(w1, f32)
    b1 = np.asarray(b1, f32)
    w2 = np.asarray(w2, f32)
    b2 = np.asarray(b2, f32)

    wq, wk, wv = in_proj_w[:H], in_proj_w[H:2 * H], in_proj_w[2 * H:]
    bq, bk, bv = in_proj_b[:H], in_proj_b[H:2 * H], in_proj_b[2 * H:]

    q = cls_embedding @ wq.T + bq                       # [H]
    qh = q.reshape(NH, DH)
    r = np.einsum("hd,hdD->hD", qh, wk.reshape(NH, DH, H))   # [NH, H]
    c_h = np.einsum("hd,hd->h", qh, bk.reshape(NH, DH))      # [NH]
    k_h = np.exp(c_h * SCALE)                                # [NH]
    kv = np.repeat(k_h, DH)                                  # [H]

    def pack(a, kc):
        # [kc*128, W] -> [128, kc*W] with row p holding chunks k at [k*W:(k+1)*W]
        w_ = a.shape[1]
        return a.reshape(kc, 128, w_).transpose(1, 0, 2).reshape(128, kc * w_).copy()

    wvT_s = pack((wv * kv[:, None]).T, KC)               # scaled V weights
    bv_s = (bv * kv)[None, :]                            # [1, H]
    rhT = pack(r.T.copy(), KC)                           # [128, KC*NH]

    k_cls = cls_embedding @ wk.T + bk
    s_cls = np.einsum("hd,hd->h", qh, k_cls.reshape(NH, DH)) * SCALE
    p_cls = np.exp(s_cls)
    v_cls = cls_embedding @ wv.T + bv
    gcls_wv = np.repeat(p_cls, DH) * v_cls               # [H]
    gclscol = gcls_wv.reshape(KC, 128).T.copy()          # [128, KC]
    pclscol = p_cls[:, None].astype(f32)                 # [NH, 1]

    w1_span, w1_w, w1_cls = w1[:, :H], w1[:, H:H + WD], w1[:, H + WD:]
    W_eff = w1_span @ out_proj_w                         # [INNER, H]
    b_eff = w1_span @ out_proj_b + b1                    # [INNER]
    TC = width_table @ w1_w.T                            # [9, INNER]
    TC_pad = np.zeros((128, INNER), f32)
    TC_pad[:MAXW + 1] = TC
    cls_bias = cls_reps @ w1_cls.T + b_eff[None, :]      # [B, INNER]

    scols = (np.arange(128, dtype=f32)[:, None]
             + 128.0 * np.arange(GC, dtype=f32)[None, :]).copy()
    kb = np.tile(k_h.astype(f32)[None, :], (128, 1)).copy()
    b2bc = np.tile(b2[None, :], (128, 1)).astype(f32)

    common = dict(
        wvT=wvT_s.astype(bf), rhT=rhT.astype(bf),
        bvbc=np.tile(bv_s, (128, 1)).astype(f32),
        scols=scols, kb=kb,
        gclscol=gclscol.astype(f32), pclscol=pclscol,
        weffT=pack(W_eff.T, KC).astype(bf), tcT=TC_pad.astype(bf),
        w2T=pack(w2.T, OC).astype(bf), b2bc=b2bc,
    )

    starts_all = span_ids[..., 0]                        # [B, N]
    widths_all = span_widths                             # [B, N]

    # dedup each batch; size the device span capacity to the largest half
    per_batch = [_dedup_batch(starts_all[b_], widths_all[b_], span_masks[b_])
                 for b_ in range(B)]
    halves = []
    gather = np.empty((B, N, 2), np.int64)               # (core, slot)
    max_half = 0
    for b_, (us, uw, ue, inv) in enumerate(per_batch):
        n_u = len(us)
        n0 = (n_u + 1) // 2
        max_half = max(max_half, n0, n_u - n0)
        halves.append(((us[:n0], uw[:n0], ue[:n0]),
                       (us[n0:], uw[n0:], ue[n0:])))
        hi = inv >= n0
        gather[b_, :, 0] = 2 * b_ + hi
        gather[b_, :, 1] = inv - n0 * hi

    blks = _blocks_for(max(128, -(-max_half // 8) * 8))
    nspc = sum(blks)

    global _GATHER, _BLKS
    _GATHER, _BLKS = gather, blks

    in_maps = []
    for core in range(8):
        b_idx, half = core // 2, core % 2
        us, uw, ue = halves[b_idx][half]
        st = np.zeros((1, nspc), f32)
        en = np.zeros((1, nspc), f32)
        wd = np.zeros((1, nspc), f32)
        st[0, :len(us)] = us
        en[0, :len(ue)] = ue
        wd[0, :len(uw)] = uw
        im = dict(common)
        im["xT"] = pack(token_reps[b_idx].T, KC).astype(bf)
        im["stbc"] = np.broadcast_to(st, (128, nspc)).astype(np.float16)
        im["enbc"] = np.broadcast_to(en, (128, nspc)).astype(np.float16)
        im["wdbc"] = np.broadcast_to(wd, (128, nspc)).astype(np.float16)
        cc = cls_bias[b_idx].reshape(OC, 128).T.copy()   # [128, OC]
        im["clscol"] = cc.astype(f32)
        in_maps.append(im)
    return in_maps


_NC_CACHE = {}
_GATHER = None
_BLKS = (512, 512, 200)


def _get_nc():
    if _BLKS not in _NC_CACHE:
        _NC_CACHE[_BLKS] = build(_BLKS)
    return _NC_CACHE[_BLKS]


def run_on_device(in_maps, **kwargs):
    from concourse.bass_utils import run_bass_kernel_spmd
    return run_bass_kernel_spmd(_get_nc(), in_maps, core_ids=list(range(8)),
                                **kwargs)


def _assemble(results):
    stacked = np.stack([np.asarray(results[c]["out"], np.float32)
                        for c in range(8)])                    # [8, nspc, H]
    out = stacked[_GATHER[..., 0], _GATHER[..., 1]]            # [B, N, H]
    return np.ascontiguousarray(out, np.float32)


def kernel(**inputs):
    in_maps = _prep_in_maps(**inputs)
    res = run_on_device(in_maps)
    return _assemble(res.results)


# revision 44
# speedup vs baseline: 1.0497x; 1.0497x over previous
"""Sparse span-attention kernel for Trainium2 (8 NeuronCores, SPMD).

Math (matches the reference):
  - Only the CLS query row of the MHA survives downstream, and K/V are
    shared by all spans of a sequence. Per batch we precompute
    P[t,h] = exp(score[t,h]) and WV[t,j] = P[t,head(j)] * v[t,j] once,
    then each span's softmax context is a masked row-sum:
      ctx[n] = (G_cls + sum_{t in span n} G[t]) split into num/den.
    The masked row-sum over 512 token positions is a mask matmul
    (mask built on VectorE from span start/end via iota compares).
  - out_proj is folded into w1 (host-side weight fusion); the width-
    embedding contribution becomes a padded [128,3072] table applied via
    a width-one-hot matmul; the cls_reps contribution is a per-batch bias.

Span dedup: a span's output depends only on (start, width, mask) — and
for masked-out spans on width alone. Random spans collide heavily
(~2420 unique of 4096 per batch), so the host dedups spans per batch,
the device computes only unique spans, and the host expands the result
back with a gather. Per-core capacity is 1280 unique spans (blocks of
512/512/256); a 2048-capacity variant is compiled on demand if an input
ever has more unique spans than that.

Sharding: core c handles batch c//2 and half c%2 of that batch's
deduped span list. No collectives: each core writes its own output
shard; the host gathers.
"""

import math

import numpy as np
import ml_dtypes

import concourse.bass as bass
import concourse.mybir as mybir
from concourse.bass import ts
from concourse.tile import TileContext
from concourse.vector_clock import ScopedClock

F32 = mybir.dt.float32
F16 = mybir.dt.float16
BF16 = mybir.dt.bfloat16
bf = ml_dtypes.bfloat16
ALU = mybir.AluOpType
ACTF = mybir.ActivationFunctionType

B, S, H, NH, MAXW = 4, 512, 768, 4, 8
DH = H // NH                # 192
N = S * MAXW                # 4096 spans per batch
INNER = 3072
WD = 64
SCALE = 1.0 / math.sqrt(DH)
KC = H // 128               # 6 contraction chunks of 128 over hidden
OC = INNER // 128           # 24 chunks over inner dim
GC = S // 128               # 4 token chunks

def _blocks_for(cap):
    """Split a span capacity into matmul-friendly blocks of <=512."""
    blks = []
    while cap > 512:
        blks.append(512)
        cap -= 512
    blks.append(cap)
    return tuple(blks)

# ---------------------------------------------------------------------------
# walrus workaround: this build rejects >1 sync wait per instruction.
# Hoist extra waits onto standalone EventSemaphore instructions.
# ---------------------------------------------------------------------------
_orig_commit = TileContext._commit_instruction


def _split_waits(self, inst):
    si = inst.sync_info
    waits = list(si.on_wait)
    for w in waits[:-1]:
        ev = mybir.InstEventSemaphore(
            name=self.nc.get_next_instruction_name(),
            engine=inst.engine,
            ins=[],
            outs=[],
            sync_info=mybir.SyncInfo(on_wait=[w], on_update=[]),
        )
        self._add_instruction(ev)
    inst.sync_info = mybir.SyncInfo(on_wait=[waits[-1]], on_update=list(si.on_update))


def _patched_commit(self, inst, lazy_reg_writes=True):
    if (
        inst.engine != mybir.EngineType.Unassigned
        and inst.sync_info is not None
        and len(inst.sync_info.on_wait) > 1
    ):
        _split_waits(self, inst)
    return _orig_commit(self, inst, lazy_reg_writes)


def _patched_drain_and_barrier(self, tick_clock, wait_clock):
    nc = self.nc
    probe = nc.sync.drain()
    wait_clock.add_sem_waits(probe.ins, ScopedClock({None: tick_clock.global_clock}))
    waits = list(probe.ins.sync_info.on_wait)
    probe.ins.sync_info = mybir.SyncInfo(on_wait=[], on_update=[])
    for w in waits:
        ev = mybir.InstEventSemaphore(
            name=nc.get_next_instruction_name(),
            engine=mybir.EngineType.SP,
            ins=[],
            outs=[],
            sync_info=mybir.SyncInfo(on_wait=[w], on_update=[]),
        )
        nc.register_instruction(ev, overwrite=True)
        nc.cur_bb.bb.add_instruction(ev)
    nc.sync.drain()

    nc.all_engine_barrier()
    assert self.sems is not None
    popped = nc._tile_sem_poison_stack.pop()
    assert popped is self._sem_poison
    nc.clear_and_free_semaphores(list(self.sems.allocated().values()))


def _install_patches():
    TileContext._commit_instruction = _patched_commit
    TileContext._drain_and_barrier = _patched_drain_and_barrier


_install_patches()


# ---------------------------------------------------------------------------
# device graph
# ---------------------------------------------------------------------------
def build(blks):
    nspc = sum(blks)
    nblk = len(blks)

    nc = bass.Bass("TRN2")

    d_xT = nc.dram_tensor("xT", [128, KC * S], BF16, kind="ExternalInput")
    d_wvT = nc.dram_tensor("wvT", [128, KC * H], BF16, kind="ExternalInput")
    d_rhT = nc.dram_tensor("rhT", [128, KC * NH], BF16, kind="ExternalInput")
    d_stbc = nc.dram_tensor("stbc", [128, nspc], F16, kind="ExternalInput")
    d_enbc = nc.dram_tensor("enbc", [128, nspc], F16, kind="ExternalInput")
    d_wdbc = nc.dram_tensor("wdbc", [128, nspc], F16, kind="ExternalInput")
    d_bvbc = nc.dram_tensor("bvbc", [128, H], F32, kind="ExternalInput")
    d_scols = nc.dram_tensor("scols", [128, GC], F32, kind="ExternalInput")
    d_kb = nc.dram_tensor("kb", [128, NH], F32, kind="ExternalInput")
    d_gclscol = nc.dram_tensor("gclscol", [128, KC], F32, kind="ExternalInput")
    d_pclscol = nc.dram_tensor("pclscol", [NH, 1], F32, kind="ExternalInput")
    d_weffT = nc.dram_tensor("weffT", [128, KC * INNER], BF16, kind="ExternalInput")
    d_clscol = nc.dram_tensor("clscol", [128, OC], F32, kind="ExternalInput")
    d_tcT = nc.dram_tensor("tcT", [128, INNER], BF16, kind="ExternalInput")
    d_w2T = nc.dram_tensor("w2T", [128, OC * H], BF16, kind="ExternalInput")
    d_b2bc = nc.dram_tensor("b2bc", [128, H], F32, kind="ExternalInput")
    d_out = nc.dram_tensor("out", [nspc, H], BF16, kind="ExternalOutput")

    with TileContext(nc) as tc:
        with tc.tile_pool(name="const", bufs=1) as cp, \
             tc.tile_pool(name="work", bufs=1) as wp, \
             tc.tile_pool(name="sbM", bufs=2) as sbM, \
             tc.tile_pool(name="sbB", bufs=2) as sbB, \
             tc.tile_pool(name="sbB1", bufs=1) as sbB1, \
             tc.tile_pool(name="sbO", bufs=2) as sbO, \
             tc.tile_pool(name="dramp", bufs=2, space="DRAM") as dramp:
            # ---- warmup tile: vector-memset so the PE can start ramping
            # immediately, with no DMA dependency
            wu_sb = cp.tile([128, 512], BF16)
            nc.vector.memset(wu_sb[:], 0.0)

            # ---- stage-A inputs (sync/HWDGE queues). Queue order matters:
            # xT first (head of the dependency chain — every small DMA ahead
            # of it costs queue time), then the small constants, then the V
            # weights.
            xt = cp.tile([128, KC, S], BF16)
            nc.sync.dma_start(xt[:], d_xT.rearrange("p (k s) -> p k s", k=KC))
            rh_sb = cp.tile([128, KC, NH], BF16)
            nc.sync.dma_start(rh_sb[:], d_rhT.rearrange("p (k h) -> p k h", k=KC))
            scols_sb = cp.tile([128, GC], F32)
            nc.sync.dma_start(scols_sb[:], d_scols[:])
            kb_sb = cp.tile([128, NH], F32)
            nc.sync.dma_start(kb_sb[:], d_kb[:])
            gclscol_sb = cp.tile([128, KC], F32)
            nc.sync.dma_start(gclscol_sb[:], d_gclscol[:])
            pclscol_sb = cp.tile([NH, 1], F32)
            nc.sync.dma_start(pclscol_sb[:], d_pclscol[:])
            clscol_sb = cp.tile([128, OC], F32)
            nc.sync.dma_start(clscol_sb[:], d_clscol[:])

            # span bounds, host-pre-broadcast to 128 partitions in f16, on
            # the Activation HWDGE queue (independent of the SP queue) so
            # the block-0 masks never wait behind the main input stream
            stbc_sb = cp.tile([128, nspc], F16)
            nc.scalar.dma_start(stbc_sb[:], d_stbc[:])
            enbc_sb = cp.tile([128, nspc], F16)
            nc.scalar.dma_start(enbc_sb[:], d_enbc[:])
            wdbc_sb = cp.tile([128, nspc], F16)
            nc.scalar.dma_start(wdbc_sb[:], d_wdbc[:])
            bvbc_sb = cp.tile([128, H], F32)
            nc.scalar.dma_start(bvbc_sb[:], d_bvbc[:])

            # ---- per-block mask tiles; block k+1's masks are built while
            # block k computes, block 0's before the weights hit the queues
            MT_all = [wp.tile([128, GC, blks[b_]], BF16, tag=f"mt{b_}",
                              name=f"mt{b_}")
                      for b_ in range(nblk)]
            OH_all = [wp.tile([128, blks[b_]], BF16, tag=f"oh{b_}",
                              name=f"oh{b_}")
                      for b_ in range(nblk)]

            def emit_masks(blk):
                bs = blks[blk]
                n0 = sum(blks[:blk])
                st_bc = stbc_sb[:, n0:n0 + bs]
                en_bc = enbc_sb[:, n0:n0 + bs]
                wd_bc = wdbc_sb[:, n0:n0 + bs]
                for c in range(GC):
                    tmp = sbM.tile([128, bs], F16, tag="tmp", name="tmp")
                    nc.vector.tensor_scalar(tmp[:], st_bc,
                                            scols_sb[:, c:c + 1], None,
                                            ALU.is_le)
                    nc.vector.scalar_tensor_tensor(
                        MT_all[blk][:, c, :], en_bc,
                        scols_sb[:, c:c + 1], tmp[:],
                        ALU.is_gt, ALU.mult)
                nc.vector.tensor_scalar(OH_all[blk][:], wd_bc,
                                        scols_sb[:, 0:1], None, ALU.is_equal)

            emit_masks(0)

            # V weights queue after the block-0 mask broadcasts
            # V weights ride the SWDGE queue (fast, otherwise only carries
            # the late-needed FFN weights), gated on the warmup memset
            wv_t = [cp.tile([128, H], BF16, tag=f"wv{k}", name=f"wv{k}")
                    for k in range(KC)]
            for k in range(KC):
                nc.vector.tensor_copy(wv_t[k][0:1, 0:1], wu_sb[0:1, 0:1])
                nc.gpsimd.dma_start(wv_t[k][:], d_wvT[:, k * H:(k + 1) * H])
            # b2 broadcast is only needed by the FFN2 epilogue; queue it last
            b2bc_sb = cp.tile([128, H], F32)
            for j0 in (0, 384):
                nc.sync.dma_start(b2bc_sb[:, j0:j0 + 384],
                                  d_b2bc[:, j0:j0 + 384])

            G = wp.tile([128, GC, H + NH], BF16)       # [token, chunk, WV|P]
            G_p = wp.tile([128, GC, NH], F32)          # raw P per token chunk

            # partition ranges of each hidden chunk -> head row
            RB_PIECES = ((0, 0, 128, 0), (1, 0, 64, 0), (1, 64, 128, 1),
                         (2, 0, 128, 1), (3, 0, 128, 2), (4, 0, 64, 2),
                         (4, 64, 128, 3), (5, 0, 128, 3))

            def emit_den_recip(blk, dpool):
                bs = blks[blk]
                MT = MT_all[blk]
                ps_d = dpool.tile([128, bs], F32, tag="d", name="d")
                for c in range(GC):
                    nc.tensor.matmul(ps_d[0:NH, :], G[:, c, H:H + NH],
                                     MT[:, c, :], start=(c == 0),
                                     stop=(c == GC - 1))
                den_sb = sbB.tile([NH, bs], F32, tag="den", name="den")
                nc.vector.tensor_scalar(den_sb[:], ps_d[0:NH, :],
                                        pclscol_sb[0:NH, 0:1], None, ALU.add)
                rec = sbB.tile([NH, bs], F16, tag="rc", name="rc")
                with nc.allow_low_precision(reason="f16 recip: 0.05% rel err, "
                                            "well inside the 2e-2 gate"):
                    nc.vector.reciprocal(rec[:], den_sb[:])
                # broadcast per-head reciprocals to the 768 hidden rows via
                # a DRAM round-trip (SBUF APs cannot partition-broadcast);
                # f16 halves the round-trip bytes
                scr = dramp.tile([NH, bs], F16, tag="rsc", name="rsc")
                nc.scalar.dma_start(scr[:], rec[:])
                rb = [sbB1.tile([128, bs], F16, tag=f"rb{c}", name=f"rb{c}")
                      for c in range(KC)]
                for (c, p0, p1, h) in RB_PIECES:
                    nc.scalar.dma_start(
                        rb[c][p0:p1, :],
                        scr[h:h + 1, :].to_broadcast((p1 - p0, bs)))
                return rb

            rbs = {}

            # ---- big weights: SWDGE queues, split per chunk so transfers
            # parallelize across rings and FFN1's k-chunk matmuls depend only
            # on their own slice; gated on the first xT chunk landing so they
            # don't starve the stage-A input DMAs of HBM bandwidth
            weff_sb = cp.tile([128, KC, INNER], BF16)
            for k in range(KC):
                nc.vector.tensor_copy(weff_sb[0:1, k, 0:1], wu_sb[0:1, 0:1])
                nc.gpsimd.dma_start(weff_sb[:, k, :],
                                    d_weffT[:, k * INNER:(k + 1) * INNER])
            tc_sb = cp.tile([128, INNER], BF16)
            nc.vector.tensor_copy(tc_sb[0:1, 0:1], xt[0:1, 0, 0:1])
            nc.gpsimd.dma_start(tc_sb[:], d_tcT[:])
            w2_sb = cp.tile([128, OC, H], BF16)
            for o in range(OC):
                nc.vector.tensor_copy(w2_sb[0:1, o, 0:1], xt[0:1, 0, 0:1])
                nc.gpsimd.dma_start(w2_sb[:, o, :],
                                    d_w2T[:, o * H:(o + 1) * H])

            # ---- stage A: warmup burst on the memset tile (ramp the PE
            # while input DMAs land), scores/exp/P, V projections + WV
            # scaling, then block-0 denominator (after V so the PE never
            # waits on the vector engine's masks/P)
            with tc.tile_pool(name="psA", bufs=1, space="PSUM") as psA, \
                 tc.tile_pool(name="psAv", bufs=3, space="PSUM") as psAv, \
                 tc.tile_pool(name="psA1", bufs=1, space="PSUM") as psA1:
                ps_wu = psAv.tile([128, H], F32, tag="v")
                for _ in range(8):
                    nc.tensor.matmul(ps_wu[:, 0:512], wu_sb[:, 0:128],
                                     wu_sb[:, 0:512], start=True, stop=True)
                ps_sc = psA.tile([128, GC, NH], F32, tag="sc")
                for c in range(GC):
                    for k in range(KC):
                        nc.tensor.matmul(ps_sc[:, c, :], xt[:, k, ts(c, 128)],
                                         rh_sb[:, k, :],
                                         start=(k == 0), stop=(k == KC - 1))
                    nc.scalar.activation(G_p[:, c, :], ps_sc[:, c, :], ACTF.Exp,
                                         scale=SCALE)
                    nc.vector.tensor_tensor(G[:, c, H:H + NH], G_p[:, c, :],
                                            kb_sb[:, :], ALU.mult)
                # block-0 denominator between scores and V: the recip DRAM
                # round-trip hides under the V projections
                rbs[0] = emit_den_recip(0, psA1)
                for c in range(GC):
                    ps_v = psAv.tile([128, H], F32, tag="v")
                    for f0, fw in ((0, 512), (512, 256)):
                        for k in range(KC):
                            nc.tensor.matmul(ps_v[:, f0:f0 + fw],
                                             xt[:, k, ts(c, 128)],
                                             wv_t[k][:, f0:f0 + fw],
                                             start=(k == 0),
                                             stop=(k == KC - 1))
                    # V bias folded in on the vector engine (a contraction-1
                    # matmul costs ~2.5x a normal row on the PE)
                    nc.vector.tensor_tensor(ps_v[:, :], ps_v[:, :],
                                            bvbc_sb[:, :], ALU.add)
                    for h in range(NH):
                        nc.vector.tensor_scalar_mul(
                            G[:, c, ts(h, DH)], ps_v[:, ts(h, DH)],
                            G_p[:, c, h:h + 1])

            # ---- stage B: per span block
            with tc.tile_pool(name="psD", bufs=1, space="PSUM") as psD, \
                 tc.tile_pool(name="psN", bufs=3, space="PSUM") as psN, \
                 tc.tile_pool(name="psH", bufs=2, space="PSUM") as psH, \
                 tc.tile_pool(name="psO", bufs=2, space="PSUM") as psO:
                for blk in range(nblk):
                    bs = blks[blk]
                    n0 = sum(blks[:blk])
                    MT = MT_all[blk]
                    OH = OH_all[blk][:]
                    rb_t = rbs.pop(blk)

                    # numerators + recip broadcast + divide (CLS via epilogue)
                    ctx_t = [sbB1.tile([128, bs], BF16, tag=f"ctx{c}",
                                       name=f"ctx{c}")
                             for c in range(KC)]
                    ps_ns = {}

                    def emit_num(c):
                        ps_n = psN.tile([128, bs], F32, tag="n")
                        for cc in range(GC):
                            nc.tensor.matmul(ps_n[:], G[:, cc, ts(c, 128)],
                                             MT[:, cc, :], start=(cc == 0),
                                             stop=(cc == GC - 1))
                        ps_ns[c] = ps_n

                    def emit_div(c):
                        nc.vector.scalar_tensor_tensor(
                            ctx_t[c][:], ps_ns.pop(c),
                            gclscol_sb[:, c:c + 1], rb_t[c][:],
                            ALU.add, ALU.mult)

                    emit_num(0)
                    emit_num(1)
                    emit_num(2)
                    for c in range(KC):
                        emit_div(c)
                        if c + 3 < KC:
                            emit_num(c + 3)
                    if blk + 1 < nblk:
                        emit_masks(blk + 1)

                    # FFN1 (out_proj folded in) + width table + cls bias, relu
                    h1_t = [sbB1.tile([128, bs], BF16, tag=f"h1_{o}",
                                      name=f"h1_{o}")
                            for o in range(OC)]
                    for o in range(OC):
                        ps_h = psH.tile([128, bs], F32, tag="h")
                        for k in range(KC):
                            nc.tensor.matmul(ps_h[:], weff_sb[:, k, ts(o, 128)],
                                             ctx_t[k][:],
                                             start=(k == 0), stop=False)
                        nc.tensor.matmul(ps_h[:], tc_sb[:, ts(o, 128)],
                                         OH, start=False, stop=True)
                        nc.scalar.activation(h1_t[o][:], ps_h[:], ACTF.Relu,
                                             bias=clscol_sb[:, o:o + 1])

                    if blk + 1 < nblk:
                        rbs[blk + 1] = emit_den_recip(blk + 1, psD)

                    # FFN2 back to [span, hidden]; b2 added in the epilogue
                    for t in range((bs + 127) // 128):
                        tw = min(128, bs - t * 128)
                        out_sb = sbO.tile([128, H], BF16, tag="os")
                        for f0, fw in ((0, 512), (512, 256)):
                            ps_o = psO.tile([128, 512], F32, tag="o")
                            for k in range(OC):
                                nc.tensor.matmul(ps_o[0:tw, 0:fw],
                                                 h1_t[k][:, t * 128:t * 128 + tw],
                                                 w2_sb[:, k, f0:f0 + fw],
                                                 start=(k == 0),
                                                 stop=(k == OC - 1))
                            nc.vector.tensor_tensor(out_sb[0:tw, f0:f0 + fw],
                                                    ps_o[0:tw, 0:fw],
                                                    b2bc_sb[0:tw, f0:f0 + fw],
                                                    ALU.add)
                        nc.sync.dma_start(
                            d_out[n0 + t * 128:n0 + t * 128 + tw, :],
                            out_sb[0:tw, :])
    return nc


# ---------------------------------------------------------------------------
# host-side prep
# ---------------------------------------------------------------------------
def _dedup_batch(starts_b, widths_b, masks_b):
    """Unique (start,width,mask) combos (mask-0 spans key on width only).

    Returns (u_starts, u_widths, u_ends, inv) with inv mapping each of the
    N spans to its row in the unique list.
    """
    s = starts_b.astype(np.int64)
    w = widths_b.astype(np.int64)
    m = masks_b.astype(np.int64)
    key = np.where(m > 0, (1 << 20) + s * 16 + w, w)
    uniq, inv = np.unique(key, return_inverse=True)
    live = uniq >= (1 << 20)
    us = np.where(live, (uniq - (1 << 20)) >> 4, 0)
    uw = np.where(live, (uniq - (1 << 20)) & 15, uniq)
    ue = np.where(live, us + uw, 0)
    return us.astype(np.float32), uw.astype(np.float32), \
        ue.astype(np.float32), inv.reshape(-1)


def _prep_in_maps(token_reps, span_ids, span_masks, cls_reps, span_widths,
                  cls_embedding, in_proj_w, in_proj_b, out_proj_w, out_proj_b,
                  width_table, w1, b1, w2, b2):
    f32 = np.float32
    token_reps = np.asarray(token_reps, f32)
    span_ids = np.asarray(span_ids)
    span_masks = np.asarray(span_masks)
    cls_reps = np.asarray(cls_reps, f32)
    span_widths = np.asarray(span_widths)
    cls_embedding = np.asarray(cls_embedding, f32)
    in_proj_w = np.asarray(in_proj_w, f32)
    in_proj_b = np.asarray(in_proj_b, f32)
    out_proj_w = np.asarray(out_proj_w, f32)
    out_proj_b = np.asarray(out_proj_b, f32)
    width_table = np.asarray(width_table, f32)
    w1 = np.asarray(w1, f32)
    b1 = np.asarray(b1, f32)
    w2 = np.asarray(w2, f32)
    b2 = np.asarray(b2, f32)

    wq, wk, wv = in_proj_w[:H], in_proj_w[H:2 * H], in_proj_w[2 * H:]
    bq, bk, bv = in_proj_b[:H], in_proj_b[H:2 * H], in_proj_b[2 * H:]

    q = cls_embedding @ wq.T + bq                       # [H]
    qh = q.reshape(NH, DH)
    r = np.einsum("hd,hdD->hD", qh, wk.reshape(NH, DH, H))   # [NH, H]
    c_h = np.einsum("hd,hd->h", qh, bk.reshape(NH, DH))      # [NH]
    k_h = np.exp(c_h * SCALE)                                # [NH]
    kv = np.repeat(k_h, DH)                                  # [H]

    def pack(a, kc):
        # [kc*128, W] -> [128, kc*W] with row p holding chunks k at [k*W:(k+1)*W]
        w_ = a.shape[1]
        return a.reshape(kc, 128, w_).transpose(1, 0, 2).reshape(128, kc * w_).copy()

    wvT_s = pack((wv * kv[:, None]).T, KC)               # scaled V weights
    bv_s = (bv * kv)[None, :]                            # [1, H]
    rhT = pack(r.T.copy(), KC)                           # [128, KC*NH]

    k_cls = cls_embedding @ wk.T + bk
    s_cls = np.einsum("hd,hd->h", qh, k_cls.reshape(NH, DH)) * SCALE
    p_cls = np.exp(s_cls)
    v_cls = cls_embedding @ wv.T + bv
    gcls_wv = np.repeat(p_cls, DH) * v_cls               # [H]
    gclscol = gcls_wv.reshape(KC, 128).T.copy()          # [128, KC]
    pclscol = p_cls[:, None].astype(f32)                 # [NH, 1]

    w1_span, w1_w, w1_cls = w1[:, :H], w1[:, H:H + WD], w1[:, H + WD:]
    W_eff = w1_span @ out_proj_w                         # [INNER, H]
    b_eff = w1_span @ out_proj_b + b1                    # [INNER]
    TC = width_table @ w1_w.T                            # [9, INNER]
    TC_pad = np.zeros((128, INNER), f32)
    TC_pad[:MAXW + 1] = TC
    cls_bias = cls_reps @ w1_cls.T + b_eff[None, :]      # [B, INNER]

    scols = (np.arange(128, dtype=f32)[:, None]
             + 128.0 * np.arange(GC, dtype=f32)[None, :]).copy()
    kb = np.tile(k_h.astype(f32)[None, :], (128, 1)).copy()
    b2bc = np.tile(b2[None, :], (128, 1)).astype(f32)

    common = dict(
        wvT=wvT_s.astype(bf), rhT=rhT.astype(bf),
        bvbc=np.tile(bv_s, (128, 1)).astype(f32),
        scols=scols, kb=kb,
        gclscol=gclscol.astype(f32), pclscol=pclscol,
        weffT=pack(W_eff.T, KC).astype(bf), tcT=TC_pad.astype(bf),
        w2T=pack(w2.T, OC).astype(bf), b2bc=b2bc,
    )

    starts_all = span_ids[..., 0]                        # [B, N]
    widths_all = span_widths                             # [B, N]

    # dedup each batch; size the device span capacity to the largest half
    per_batch = [_dedup_batch(starts_all[b_], widths_all[b_], span_masks[b_])
                 for b_ in range(B)]
    halves = []
    gather = np.empty((B, N, 2), np.int64)               # (core, slot)
    max_half = 0
    for b_, (us, uw, ue, inv) in enumerate(per_batch):
        n_u = len(us)
        n0 = (n_u + 1) // 2
        max_half = max(max_half, n0, n_u - n0)
        halves.append(((us[:n0], uw[:n0], ue[:n0]),
                       (us[n0:], uw[n0:], ue[n0:])))
        hi = inv >= n0
        gather[b_, :, 0] = 2 * b_ + hi
        gather[b_, :, 1] = inv - n0 * hi

    blks = _blocks_for(max(128, -(-max_half // 8) * 8))
    nspc = sum(blks)

    global _GATHER, _BLKS
    _GATHER, _BLKS = gather, blks

    in_maps = []
    for core in range(8):
        b_idx, half = core // 2, core % 2
        us, uw, ue = halves[b_idx][half]
        st = np.zeros((1, nspc), f32)
        en = np.zeros((1, nspc), f32)
        wd = np.zeros((1, nspc), f32)
        st[0, :len(us)] = us
        en[0, :len(ue)] = ue
        wd[0, :len(uw)] = uw
        im = dict(common)
        im["xT"] = pack(token_reps[b_idx].T, KC).astype(bf)
        im["stbc"] = np.broadcast_to(st, (128, nspc)).astype(np.float16)
        im["enbc"] = np.broadcast_to(en, (128, nspc)).astype(np.float16)
        im["wdbc"] = np.broadcast_to(wd, (128, nspc)).astype(np.float16)
        cc = cls_bias[b_idx].reshape(OC, 128).T.copy()   # [128, OC]
        im["clscol"] = cc.astype(f32)
        in_maps.append(im)
    return in_maps


_NC_CACHE = {}
_GATHER = None
_BLKS = (512, 512, 200)


def _get_nc():
    if _BLKS not in _NC_CACHE:
        _NC_CACHE[_BLKS] = build(_BLKS)
    return _NC_CACHE[_BLKS]


def run_on_device(in_maps, **kwargs):
    from concourse.bass_utils import run_bass_kernel_spmd
    return run_bass_kernel_spmd(_get_nc(), in_maps, core_ids=list(range(8)),
                                **kwargs)


def _assemble(results):
    stacked = np.stack([np.asarray(results[c]["out"], np.float32)
                        for c in range(8)])                    # [8, nspc, H]
    out = stacked[_GATHER[..., 0], _GATHER[..., 1]]            # [B, N, H]
    return np.ascontiguousarray(out, np.float32)


def kernel(**inputs):
    in_maps = _prep_in_maps(**inputs)
    res = run_on_device(in_maps)
    return _assemble(res.results)
